# revision 6
# baseline (speedup 1.0000x reference)
"""Trainium2 Bass kernel for the batched Kalman filter problem.

Problem: G=2048 groups, T=256 steps, M=8 obs dims, S=16 state dims.
Output: means [G,T,M], covs [G,T,M,M] (the per-step predicted measurement
distribution).

Structure exploited:
  * The covariance recursion P_t is data-independent and P0 is broadcast, so
    P_t / the Kalman gain / the output covariance ycov_t are identical for
    every group.  ycov [T,M,M] is computed on host (tiny, sequential 16x16
    math) and the device broadcasts it into the [G,T,M,M] output — that
    broadcast write (134 MB) is the memory-bound bulk of the work.
  * The mean recursion is affine with shared coefficient matrices:
        mean_{t+1} = mean_t @ A_t + obs_t @ B_t,   ymean_t = mean_t @ H^T
    Chunking t into blocks of C=16 turns this into a short sequential chain
    of dense matmuls over the whole group batch (device, TensorE).

Sharding: groups split across 8 cores (256 groups/core), pure data parallel.

Device per-core layout (all f32):
  obs_pack  [128, 16*256] in  : col block k = obsT rows [128k:128k+128)
                                (obsT[(t*M+m), g] = input[g, t, m])
  w_pack    [128, 16*128] in  : col block k = W_k   (lhsT, K=(j,m') x N=(i,m))
  u_pack    [16,  16*128] in  : col block k = U_k   (lhsT, K=s x N=(i,m))
  acar_pack [16,  16*16]  in  : col block k = Acar_k (lhsT)
  bcar_pack [128, 16*16]  in  : col block k = Bcar_k (lhsT)
  mean0_t   [16, 256]     in  : initial mean broadcast over groups (meanT)
  ycov_row  [1, 16384]    in  : ycovs [T*M*M] flattened
  ymeant_out [2048, 256]  out : ymeanT[(t*M+m), g]  (host transposes back)
  covs_out   [256, 16384] out : per-group covs rows (identical), g-major
"""

import numpy as np

G, T, M, S = 2048, 256, 8, 16
NCORES = 8
G_SH = G // NCORES            # 256 groups per core
C = 16                        # time chunk
NCH = T // C                  # 16 chunks
CM = C * M                    # 128 = rows/cols of W per chunk
P = 128
YCN = T * M * M               # 16384 floats of ycov
REP_TILES = 4                 # split replicated ycov into 4 [128, 4096] tiles
REP_W = YCN // REP_TILES      # 4096

_CACHE: dict = {}


def _install_ntff_hook():
    """Provide antenv.axon_hooks (absent in this image) so bass_utils can
    NTFF-profile under axon when trace=True.  No-op if already present."""
    import sys
    import types
    import contextlib

    try:
        import antenv.axon_hooks  # noqa: F401
        return
    except ImportError:
        pass

    mod = types.ModuleType("antenv.axon_hooks")
    state = {"hook": None}
    mod.set_axon_ntff_profile_hook = lambda h: state.__setitem__("hook", h)
    mod.get_axon_ntff_profile_hook = lambda: state["hook"]
    sys.modules["antenv.axon_hooks"] = mod
    try:
        import antenv
        antenv.axon_hooks = mod
    except ImportError:
        pass

    so_path = "/opt/axon/libaxon_pjrt.so"
    try:
        import ctypes
        lib = ctypes.CDLL(so_path)
        if not hasattr(lib, "axon_start_nrt_profile"):
            return
        lib.axon_start_nrt_profile.argtypes = [
            ctypes.POINTER(ctypes.c_int64), ctypes.c_size_t]
        lib.axon_start_nrt_profile.restype = ctypes.c_int64
        lib.axon_stop_nrt_profile.argtypes = [ctypes.c_char_p]
        lib.axon_stop_nrt_profile.restype = ctypes.c_int64

        @contextlib.contextmanager
        def _hook(output_dir, device_ids):
            import jax
            jax.devices()
            if device_ids:
                ids = (ctypes.c_int64 * len(device_ids))(*device_ids)
                rc = lib.axon_start_nrt_profile(ids, len(device_ids))
            else:
                rc = lib.axon_start_nrt_profile(None, 0)
            if rc != 0:
                raise RuntimeError(f"axon_start_nrt_profile rc={rc}")
            try:
                yield
            finally:
                n = lib.axon_stop_nrt_profile(str(output_dir).encode())
                import sys as _s
                print(f"ntff profile: {n} file(s) -> {output_dir}", file=_s.stderr)

        state["hook"] = _hook
    except OSError:
        pass


_install_ntff_hook()


# ----------------------------------------------------------------------------
# Host-side math: tiny data-independent Kalman recursions (float64).
# ----------------------------------------------------------------------------

def _compute_params(init_state_mean, init_log_diag, init_off_diag, F, H, L_Q, L_R):
    F = np.asarray(F, np.float64)
    H = np.asarray(H, np.float64)
    L_Q = np.asarray(L_Q, np.float64)
    L_R = np.asarray(L_R, np.float64)
    Q = L_Q @ L_Q.T + 1e-4 * np.eye(S)
    R = L_R @ L_R.T + 1e-4 * np.eye(M)
    L0 = np.zeros((S, S))
    r, c = np.tril_indices(S, -1)
    L0[r, c] = np.asarray(init_off_diag, np.float64)
    L0 += np.diag(np.exp(np.asarray(init_log_diag, np.float64)))
    Pm = L0 @ L0.T

    I_S = np.eye(S)
    ycovs = np.zeros((T, M, M))
    A = np.zeros((T - 1, S, S))
    B = np.zeros((T - 1, M, S))
    ycovs[0] = H @ Pm @ H.T + R
    for j in range(T - 1):
        HP = H @ Pm
        Smat = HP @ H.T + R
        Kt = np.linalg.solve(Smat, HP)          # [M,S] = K^T
        P_u = Pm - Kt.T @ HP
        A[j] = (I_S - H.T @ Kt) @ F.T
        B[j] = Kt @ F.T
        Pm = F @ P_u @ F.T + Q
        ycovs[j + 1] = H @ Pm @ H.T + R

    Ht = H.T

    def getA(j):
        return A[j] if j < T - 1 else I_S

    def getB(j):
        return B[j] if j < T - 1 else np.zeros((M, S))

    U = np.zeros((NCH, S, CM))
    W = np.zeros((NCH, CM, CM))
    Acar = np.zeros((NCH, S, S))
    Bcar = np.zeros((NCH, CM, S))
    for k in range(NCH):
        t0 = k * C
        pref = I_S.copy()
        for i in range(C):
            U[k, :, i * M:(i + 1) * M] = pref @ Ht
            pref = pref @ getA(t0 + i)
        Acar[k] = pref
        for j in range(C):
            Bj = getB(t0 + j)
            mid = I_S.copy()
            for i in range(j + 1, C):
                W[k, j * M:(j + 1) * M, i * M:(i + 1) * M] = Bj @ mid @ Ht
                mid = mid @ getA(t0 + i)
            Bcar[k, j * M:(j + 1) * M, :] = Bj @ mid

    def pack(x):  # [NCH, p, n] -> [p, NCH*n], col block k = x[k]
        return np.ascontiguousarray(
            x.transpose(1, 0, 2).reshape(x.shape[1], -1)).astype(np.float32)

    mean0 = np.asarray(init_state_mean, np.float64)
    return {
        "w_pack": pack(W),                       # [128, 2048]
        "u_pack": pack(U),                       # [16, 2048]
        "acar_pack": pack(Acar),                 # [16, 256]
        "bcar_pack": pack(Bcar),                 # [128, 256]
        "mean0_t": np.ascontiguousarray(
            np.broadcast_to(mean0[:, None], (S, G_SH))).astype(np.float32),
        "ycov_row": ycovs.reshape(1, YCN).astype(np.float32),
    }


# ----------------------------------------------------------------------------
# Device kernel (Bass/Tile), SPMD over 8 cores.
# ----------------------------------------------------------------------------

def _build_nc():
    import os
    import concourse.tile as tile
    from concourse import bacc, mybir

    use_f32r = os.environ.get("KF_F32R", "1") == "1"
    use_packy = os.environ.get("KF_PACKY", "1") == "1"
    F32 = mybir.dt.float32
    F32R = mybir.dt.float32r if use_f32r else F32
    nc = bacc.Bacc("TRN2", target_bir_lowering=False, debug=False,
                   num_devices=NCORES)

    obs_d = nc.dram_tensor("obs_pack", [P, NCH * G_SH], F32, kind="ExternalInput").ap()
    w_d = nc.dram_tensor("w_pack", [P, NCH * CM], F32, kind="ExternalInput").ap()
    u_d = nc.dram_tensor("u_pack", [S, NCH * CM], F32, kind="ExternalInput").ap()
    ac_d = nc.dram_tensor("acar_pack", [S, NCH * S], F32, kind="ExternalInput").ap()
    bc_d = nc.dram_tensor("bcar_pack", [P, NCH * S], F32, kind="ExternalInput").ap()
    m0_d = nc.dram_tensor("mean0_t", [S, G_SH], F32, kind="ExternalInput").ap()
    yc_d = nc.dram_tensor("ycov_row", [1, YCN], F32, kind="ExternalInput").ap()
    ym_d = nc.dram_tensor("ymeant_out", [T * M, G_SH], F32, kind="ExternalOutput").ap()
    cv_d = nc.dram_tensor("covs_out", [G_SH, YCN], F32, kind="ExternalOutput").ap()

    with tile.TileContext(nc) as tc:
        with tc.tile_pool(name="const", bufs=1) as cpool, \
             tc.tile_pool(name="work", bufs=2) as wpool, \
             tc.tile_pool(name="mts", bufs=3) as mpool, \
             tc.tile_pool(name="ypsum", bufs=2, space="PSUM") as ypsum, \
             tc.tile_pool(name="mpsum", bufs=2, space="PSUM") as mpsum:

            # ---- ycov in, then replicate across partitions ASAP ----
            ycv = cpool.tile([1, YCN], F32, tag="ycv")
            nc.sync.dma_start(out=ycv[:], in_=yc_d)

            # matmul params (float32r tiles need a casting DMA -> gpsimd)
            _pdma = nc.gpsimd.dma_start if use_f32r else nc.sync.dma_start
            w_sb = cpool.tile([P, NCH * CM], F32R, tag="w")
            _pdma(out=w_sb[:], in_=w_d)
            u_sb = cpool.tile([S, NCH * CM], F32R, tag="u")
            _pdma(out=u_sb[:], in_=u_d)
            ac_sb = cpool.tile([S, NCH * S], F32R, tag="ac")
            _pdma(out=ac_sb[:], in_=ac_d)
            bc_sb = cpool.tile([P, NCH * S], F32R, tag="bc")
            _pdma(out=bc_sb[:], in_=bc_d)
            mt = cpool.tile([S, G_SH], F32R, tag="mt0")
            _pdma(out=mt[:], in_=m0_d)

            obs_sb = []
            for j in range(4):
                ob = cpool.tile([P, 4 * G_SH], F32R, tag=f"obs{j}")
                _pdma(out=ob[:], in_=obs_d[:, j * 4 * G_SH:(j + 1) * 4 * G_SH])
                obs_sb.append(ob)

            # replicate ycov [1, 16384] across 128 partitions, 4 tiles.
            # tiles 0/1: DVE stream_shuffle (seed rows 0/32/64/96 by DMA, then
            # broadcast row 0 within each 32-partition quadrant).
            # tiles 2/3: gpsimd partition_broadcast (runs concurrently).
            rep_tiles = []
            for j in range(REP_TILES):
                rep = cpool.tile([P, REP_W], F32, tag=f"rep{j}")
                ysl = yc_d[0:1, j * REP_W:(j + 1) * REP_W]
                if j < 2:
                    for r in (0, 32, 64, 96):
                        nc.sync.dma_start(out=rep[r:r + 1, :], in_=ysl)
                    nc.vector.stream_shuffle(rep[:], rep[:], [0] * 32)
                else:
                    nc.gpsimd.partition_broadcast(
                        rep[:], ycv[0:1, j * REP_W:(j + 1) * REP_W])
                rep_tiles.append(rep)
                for gh in range(2):
                    nc.sync.dma_start(
                        out=cv_d[gh * P:(gh + 1) * P, j * REP_W:(j + 1) * REP_W],
                        in_=rep[:])

            # ---- mean scan: 4 chunks per packed means-out DMA ----
            ym_v = ym_d.rearrange("(j c p) g -> j c p g", c=4, p=P)  # [4,4,128,256]
            for j in range(4):
                ypack = wpool.tile([P, 4 * G_SH], F32, tag="ypack")
                for k in range(4 * j, 4 * j + 4):
                    c = k % 4
                    obs_rhs = obs_sb[k // 4][:, c * G_SH:(c + 1) * G_SH]
                    py = ypsum.tile([P, G_SH], F32, tag="py")
                    nc.tensor.matmul(out=py[:], lhsT=w_sb[:, k * CM:(k + 1) * CM],
                                     rhs=obs_rhs, start=True, stop=False)
                    nc.tensor.matmul(out=py[:], lhsT=u_sb[:, k * CM:(k + 1) * CM],
                                     rhs=mt[:], start=False, stop=True)
                    if k % 2 == 0:
                        nc.vector.tensor_copy(out=ypack[:, c * G_SH:(c + 1) * G_SH], in_=py[:])
                    else:
                        nc.scalar.copy(out=ypack[:, c * G_SH:(c + 1) * G_SH], in_=py[:])

                    if k < NCH - 1:
                        pm = mpsum.tile([S, G_SH], F32, tag="pm")
                        nc.tensor.matmul(out=pm[:], lhsT=ac_sb[:, k * S:(k + 1) * S],
                                         rhs=mt[:], start=True, stop=False)
                        nc.tensor.matmul(out=pm[:], lhsT=bc_sb[:, k * S:(k + 1) * S],
                                         rhs=obs_rhs, start=False, stop=True)
                        mt2 = mpool.tile([S, G_SH], F32R, tag="mt")
                        nc.vector.tensor_copy(out=mt2[:], in_=pm[:])
                        mt = mt2
                if use_packy:
                    # dst iterated [p][c][g] to match the SBUF src's natural
                    # [p][(c g)] order (SBUF partition dim must stay first)
                    nc.sync.dma_start(
                        out=ym_d.rearrange("(jj c p) g -> jj p c g", c=4, p=P)[j],
                        in_=ypack[:])
                else:
                    for c in range(4):
                        nc.sync.dma_start(
                            out=ym_v[j, c],
                            in_=ypack[:, c * G_SH:(c + 1) * G_SH])

    nc.compile()
    return nc


def _get_nc():
    if "nc" not in _CACHE:
        _CACHE["nc"] = _build_nc()
    return _CACHE["nc"]


# ----------------------------------------------------------------------------
# Entry point
# ----------------------------------------------------------------------------

def kernel(input, init_state_mean, init_log_diag, init_off_diag, F, H, L_Q, L_R,
           **run_kwargs):
    from concourse.bass_utils import run_bass_kernel_spmd

    params = _compute_params(init_state_mean, init_log_diag, init_off_diag,
                             F, H, L_Q, L_R)

    inp = np.ascontiguousarray(np.asarray(input, np.float32))
    # obsT[(t*M+m), g], then per-core pack: [16, 128, G_SH] -> [128, 16*G_SH]
    obsT = inp.transpose(1, 2, 0).reshape(T * M, G)

    in_maps = []
    for i in range(NCORES):
        shard = obsT[:, i * G_SH:(i + 1) * G_SH]
        obs_pack = np.ascontiguousarray(
            shard.reshape(NCH, P, G_SH).transpose(1, 0, 2).reshape(P, NCH * G_SH))
        m = dict(params)
        m["obs_pack"] = obs_pack
        in_maps.append(m)

    nc = _get_nc()
    res = run_bass_kernel_spmd(nc, in_maps, core_ids=list(range(NCORES)),
                               **run_kwargs)

    means = np.empty((G, T, M), np.float32)
    covs = np.empty((G, T, M, M), np.float32)
    for i in range(NCORES):
        out = res.results[i]
        means[i * G_SH:(i + 1) * G_SH] = out["ymeant_out"].T.reshape(G_SH, T, M)
        covs[i * G_SH:(i + 1) * G_SH] = out["covs_out"].reshape(G_SH, T, M, M)
    if run_kwargs:
        _CACHE["last_results"] = res
    return means, covs


# revision 7
# speedup vs baseline: 1.2389x; 1.2389x over previous
"""Trainium2 Bass kernel for the batched Kalman filter problem.

Problem: G=2048 groups, T=256 steps, M=8 obs dims, S=16 state dims.
Output: means [G,T,M], covs [G,T,M,M] (the per-step predicted measurement
distribution).

Structure exploited:
  * The covariance recursion P_t is data-independent and P0 is broadcast, so
    P_t / the Kalman gain / the output covariance ycov_t are identical for
    every group.  ycov [T,M,M] is computed on host (tiny, sequential 16x16
    math) and the device broadcasts it into the [G,T,M,M] output — that
    broadcast write (134 MB) is the memory-bound bulk of the work.
  * The mean recursion is affine with shared coefficient matrices:
        mean_{t+1} = mean_t @ A_t + obs_t @ B_t,   ymean_t = mean_t @ H^T
    Chunking t into blocks of C=16 turns this into a short sequential chain
    of dense matmuls over the whole group batch (device, TensorE).

Sharding: groups split across 8 cores (256 groups/core), pure data parallel.

Device per-core layout (all f32):
  obs_pack  [128, 16*256] in  : col block k = obsT rows [128k:128k+128)
                                (obsT[(t*M+m), g] = input[g, t, m])
  w_pack    [128, 16*128] in  : col block k = W_k   (lhsT, K=(j,m') x N=(i,m))
  u_pack    [16,  16*128] in  : col block k = U_k   (lhsT, K=s x N=(i,m))
  acar_pack [16,  16*16]  in  : col block k = Acar_k (lhsT)
  bcar_pack [128, 16*16]  in  : col block k = Bcar_k (lhsT)
  mean0_t   [16, 256]     in  : initial mean broadcast over groups (meanT)
  ycov_row  [1, 16384]    in  : ycovs [T*M*M] flattened
  ymeant_out [2048, 256]  out : ymeanT[(t*M+m), g]  (host transposes back)
  covs_out   [256, 16384] out : per-group covs rows (identical), g-major
"""

import numpy as np

G, T, M, S = 2048, 256, 8, 16
NCORES = 8
G_SH = G // NCORES            # 256 groups per core
C = 16                        # time chunk
NCH = T // C                  # 16 chunks
CM = C * M                    # 128 = rows/cols of W per chunk
P = 128
YCN = T * M * M               # 16384 floats of ycov
REP_TILES = 4                 # split replicated ycov into 4 [128, 4096] tiles
REP_W = YCN // REP_TILES      # 4096

_CACHE: dict = {}


def _install_ntff_hook():
    """Provide antenv.axon_hooks (absent in this image) so bass_utils can
    NTFF-profile under axon when trace=True.  No-op if already present."""
    import sys
    import types
    import contextlib

    try:
        import antenv.axon_hooks  # noqa: F401
        return
    except ImportError:
        pass

    mod = types.ModuleType("antenv.axon_hooks")
    state = {"hook": None}
    mod.set_axon_ntff_profile_hook = lambda h: state.__setitem__("hook", h)
    mod.get_axon_ntff_profile_hook = lambda: state["hook"]
    sys.modules["antenv.axon_hooks"] = mod
    try:
        import antenv
        antenv.axon_hooks = mod
    except ImportError:
        pass

    so_path = "/opt/axon/libaxon_pjrt.so"
    try:
        import ctypes
        lib = ctypes.CDLL(so_path)
        if not hasattr(lib, "axon_start_nrt_profile"):
            return
        lib.axon_start_nrt_profile.argtypes = [
            ctypes.POINTER(ctypes.c_int64), ctypes.c_size_t]
        lib.axon_start_nrt_profile.restype = ctypes.c_int64
        lib.axon_stop_nrt_profile.argtypes = [ctypes.c_char_p]
        lib.axon_stop_nrt_profile.restype = ctypes.c_int64

        @contextlib.contextmanager
        def _hook(output_dir, device_ids):
            import jax
            jax.devices()
            if device_ids:
                ids = (ctypes.c_int64 * len(device_ids))(*device_ids)
                rc = lib.axon_start_nrt_profile(ids, len(device_ids))
            else:
                rc = lib.axon_start_nrt_profile(None, 0)
            if rc != 0:
                raise RuntimeError(f"axon_start_nrt_profile rc={rc}")
            try:
                yield
            finally:
                n = lib.axon_stop_nrt_profile(str(output_dir).encode())
                import sys as _s
                print(f"ntff profile: {n} file(s) -> {output_dir}", file=_s.stderr)

        state["hook"] = _hook
    except OSError:
        pass


_install_ntff_hook()


# ----------------------------------------------------------------------------
# Host-side math: tiny data-independent Kalman recursions (float64).
# ----------------------------------------------------------------------------

def _compute_params(init_state_mean, init_log_diag, init_off_diag, F, H, L_Q, L_R):
    F = np.asarray(F, np.float64)
    H = np.asarray(H, np.float64)
    L_Q = np.asarray(L_Q, np.float64)
    L_R = np.asarray(L_R, np.float64)
    Q = L_Q @ L_Q.T + 1e-4 * np.eye(S)
    R = L_R @ L_R.T + 1e-4 * np.eye(M)
    L0 = np.zeros((S, S))
    r, c = np.tril_indices(S, -1)
    L0[r, c] = np.asarray(init_off_diag, np.float64)
    L0 += np.diag(np.exp(np.asarray(init_log_diag, np.float64)))
    Pm = L0 @ L0.T

    I_S = np.eye(S)
    ycovs = np.zeros((T, M, M))
    A = np.zeros((T - 1, S, S))
    B = np.zeros((T - 1, M, S))
    ycovs[0] = H @ Pm @ H.T + R
    for j in range(T - 1):
        HP = H @ Pm
        Smat = HP @ H.T + R
        Kt = np.linalg.solve(Smat, HP)          # [M,S] = K^T
        P_u = Pm - Kt.T @ HP
        A[j] = (I_S - H.T @ Kt) @ F.T
        B[j] = Kt @ F.T
        Pm = F @ P_u @ F.T + Q
        ycovs[j + 1] = H @ Pm @ H.T + R

    Ht = H.T

    def getA(j):
        return A[j] if j < T - 1 else I_S

    def getB(j):
        return B[j] if j < T - 1 else np.zeros((M, S))

    U = np.zeros((NCH, S, CM))
    W = np.zeros((NCH, CM, CM))
    Acar = np.zeros((NCH, S, S))
    Bcar = np.zeros((NCH, CM, S))
    for k in range(NCH):
        t0 = k * C
        pref = I_S.copy()
        for i in range(C):
            U[k, :, i * M:(i + 1) * M] = pref @ Ht
            pref = pref @ getA(t0 + i)
        Acar[k] = pref
        for j in range(C):
            Bj = getB(t0 + j)
            mid = I_S.copy()
            for i in range(j + 1, C):
                W[k, j * M:(j + 1) * M, i * M:(i + 1) * M] = Bj @ mid @ Ht
                mid = mid @ getA(t0 + i)
            Bcar[k, j * M:(j + 1) * M, :] = Bj @ mid

    def pack(x):  # [NCH, p, n] -> [p, NCH*n], col block k = x[k]
        return np.ascontiguousarray(
            x.transpose(1, 0, 2).reshape(x.shape[1], -1)).astype(np.float32)

    mean0 = np.asarray(init_state_mean, np.float64)
    return {
        "w_pack": pack(W),                       # [128, 2048]
        "u_pack": pack(U),                       # [16, 2048]
        "acar_pack": pack(Acar),                 # [16, 256]
        "bcar_pack": pack(Bcar),                 # [128, 256]
        "mean0_t": np.ascontiguousarray(
            np.broadcast_to(mean0[:, None], (S, G_SH))).astype(np.float32),
        "ycov_row": ycovs.reshape(1, YCN).astype(np.float32),
    }


# ----------------------------------------------------------------------------
# Device kernel (Bass/Tile), SPMD over 8 cores.
# ----------------------------------------------------------------------------

def _build_nc():
    import os
    import concourse.tile as tile
    from concourse import bacc, mybir

    use_f32r = os.environ.get("KF_F32R", "1") == "1"
    use_packy = os.environ.get("KF_PACKY", "1") == "1"
    F32 = mybir.dt.float32
    F32R = mybir.dt.float32r if use_f32r else F32
    nc = bacc.Bacc("TRN2", target_bir_lowering=False, debug=False,
                   num_devices=NCORES)

    # matmul operands are declared float32r in DRAM directly (same bytes as
    # f32 host arrays) so the fast sync/HWDGE DMA path loads them un-cast.
    obs_d = nc.dram_tensor("obs_pack", [P, NCH * G_SH], F32R, kind="ExternalInput").ap()
    w_d = nc.dram_tensor("w_pack", [P, NCH * CM], F32R, kind="ExternalInput").ap()
    u_d = nc.dram_tensor("u_pack", [S, NCH * CM], F32R, kind="ExternalInput").ap()
    ac_d = nc.dram_tensor("acar_pack", [S, NCH * S], F32R, kind="ExternalInput").ap()
    bc_d = nc.dram_tensor("bcar_pack", [P, NCH * S], F32R, kind="ExternalInput").ap()
    m0_d = nc.dram_tensor("mean0_t", [S, G_SH], F32R, kind="ExternalInput").ap()
    yc_d = nc.dram_tensor("ycov_row", [1, YCN], F32, kind="ExternalInput").ap()
    ym_d = nc.dram_tensor("ymeant_out", [T * M, G_SH], F32, kind="ExternalOutput").ap()
    cv_d = nc.dram_tensor("covs_out", [G_SH, YCN], F32, kind="ExternalOutput").ap()

    with tile.TileContext(nc) as tc:
        with tc.tile_pool(name="const", bufs=1) as cpool, \
             tc.tile_pool(name="work", bufs=2) as wpool, \
             tc.tile_pool(name="mts", bufs=3) as mpool, \
             tc.tile_pool(name="ypsum", bufs=2, space="PSUM") as ypsum, \
             tc.tile_pool(name="mpsum", bufs=2, space="PSUM") as mpsum:

            # ---- inputs: scan-critical params first, then obs, then ycov ----
            ycv = cpool.tile([1, YCN], F32, tag="ycv")
            nc.sync.dma_start(out=ycv[:], in_=yc_d)
            mt = cpool.tile([S, G_SH], F32R, tag="mt0")
            nc.sync.dma_start(out=mt[:], in_=m0_d)
            ac_sb = cpool.tile([S, NCH * S], F32R, tag="ac")
            nc.sync.dma_start(out=ac_sb[:], in_=ac_d)
            bc_sb = cpool.tile([P, NCH * S], F32R, tag="bc")
            nc.sync.dma_start(out=bc_sb[:], in_=bc_d)
            w_sb = cpool.tile([P, NCH * CM], F32R, tag="w")
            nc.sync.dma_start(out=w_sb[:], in_=w_d)
            u_sb = cpool.tile([S, NCH * CM], F32R, tag="u")
            nc.sync.dma_start(out=u_sb[:], in_=u_d)

            obs_sb = []
            for j in range(4):
                ob = cpool.tile([P, 4 * G_SH], F32R, tag=f"obs{j}")
                nc.sync.dma_start(out=ob[:], in_=obs_d[:, j * 4 * G_SH:(j + 1) * 4 * G_SH])
                obs_sb.append(ob)

            # replicate ycov [1,16384] across 128 partitions on gpsimd (idle
            # engine; Vector stays free for the scan's carry-chain casts),
            # covs block DMAs chase each finished tile.
            for j in range(REP_TILES):
                rep = cpool.tile([P, REP_W], F32, tag=f"rep{j}")
                nc.gpsimd.partition_broadcast(
                    rep[:], ycv[0:1, j * REP_W:(j + 1) * REP_W])
                for gh in range(2):
                    nc.sync.dma_start(
                        out=cv_d[gh * P:(gh + 1) * P, j * REP_W:(j + 1) * REP_W],
                        in_=rep[:])

            # ---- mean scan: carry chain first per chunk; 4 chunks per
            # packed means-out DMA ----
            ym_v = ym_d.rearrange("(j c p) g -> j c p g", c=4, p=P)  # [4,4,128,256]
            for j in range(4):
                ypack = wpool.tile([P, 4 * G_SH], F32, tag="ypack")
                for k in range(4 * j, 4 * j + 4):
                    c = k % 4
                    obs_rhs = obs_sb[k // 4][:, c * G_SH:(c + 1) * G_SH]
                    mt_prev = mt
                    if k < NCH - 1:
                        pm = mpsum.tile([S, G_SH], F32, tag="pm")
                        nc.tensor.matmul(out=pm[:], lhsT=ac_sb[:, k * S:(k + 1) * S],
                                         rhs=mt_prev[:], start=True, stop=False)
                        nc.tensor.matmul(out=pm[:], lhsT=bc_sb[:, k * S:(k + 1) * S],
                                         rhs=obs_rhs, start=False, stop=True)
                        mt2 = mpool.tile([S, G_SH], F32R, tag="mt")
                        nc.vector.tensor_copy(out=mt2[:], in_=pm[:])
                        mt = mt2

                    py = ypsum.tile([P, G_SH], F32, tag="py")
                    nc.tensor.matmul(out=py[:], lhsT=w_sb[:, k * CM:(k + 1) * CM],
                                     rhs=obs_rhs, start=True, stop=False)
                    nc.tensor.matmul(out=py[:], lhsT=u_sb[:, k * CM:(k + 1) * CM],
                                     rhs=mt_prev[:], start=False, stop=True)
                    nc.scalar.copy(out=ypack[:, c * G_SH:(c + 1) * G_SH], in_=py[:])
                if use_packy:
                    # dst iterated [p][c][g] to match the SBUF src's natural
                    # [p][(c g)] order (SBUF partition dim must stay first)
                    nc.sync.dma_start(
                        out=ym_d.rearrange("(jj c p) g -> jj p c g", c=4, p=P)[j],
                        in_=ypack[:])
                else:
                    for c in range(4):
                        nc.sync.dma_start(
                            out=ym_v[j, c],
                            in_=ypack[:, c * G_SH:(c + 1) * G_SH])

    nc.compile()
    return nc


def _get_nc():
    if "nc" not in _CACHE:
        _CACHE["nc"] = _build_nc()
    return _CACHE["nc"]


# ----------------------------------------------------------------------------
# Entry point
# ----------------------------------------------------------------------------

def kernel(input, init_state_mean, init_log_diag, init_off_diag, F, H, L_Q, L_R,
           **run_kwargs):
    from concourse.bass_utils import run_bass_kernel_spmd

    params = _compute_params(init_state_mean, init_log_diag, init_off_diag,
                             F, H, L_Q, L_R)

    inp = np.ascontiguousarray(np.asarray(input, np.float32))
    # obsT[(t*M+m), g], then per-core pack: [16, 128, G_SH] -> [128, 16*G_SH]
    obsT = inp.transpose(1, 2, 0).reshape(T * M, G)

    in_maps = []
    for i in range(NCORES):
        shard = obsT[:, i * G_SH:(i + 1) * G_SH]
        obs_pack = np.ascontiguousarray(
            shard.reshape(NCH, P, G_SH).transpose(1, 0, 2).reshape(P, NCH * G_SH))
        m = dict(params)
        m["obs_pack"] = obs_pack
        in_maps.append(m)

    nc = _get_nc()
    res = run_bass_kernel_spmd(nc, in_maps, core_ids=list(range(NCORES)),
                               **run_kwargs)

    means = np.empty((G, T, M), np.float32)
    covs = np.empty((G, T, M, M), np.float32)
    for i in range(NCORES):
        out = res.results[i]
        means[i * G_SH:(i + 1) * G_SH] = out["ymeant_out"].T.reshape(G_SH, T, M)
        covs[i * G_SH:(i + 1) * G_SH] = out["covs_out"].reshape(G_SH, T, M, M)
    if run_kwargs:
        _CACHE["last_results"] = res
    return means, covs
